# revision 15
# baseline (speedup 1.0000x reference)
"""Trainium2 Bass kernel for CustomBertSelfAttention (no head split).

reference:
    q = hs @ Wq + bq; k = hs @ Wk + bk; v = hs @ Wv + bv        # [B,S,D]
    scores = (q @ k^T) / sqrt(64) + mask                         # [B,S,S]
    probs  = softmax(scores, -1)
    out    = probs @ v                                           # [B,S,D]

B=8, S=2048, D=1024.  Sharding: data-parallel over batch, one batch
element per NeuronCore (8 cores), no collectives.

v3 plan -- algebraic restructure on top of the all-16-bit v2:
  * scores = q k^T = hs (Wq Wk^T) hs^T + per-t/per-s bias terms.
    Precompute A = Wq Wk^T once per core (1024^3 matmul, 27.6us)
    and drop the entire K projection (55.3us): scores^T[t,s] =
    hsT^T_chunks @ (A^T hsT) with hsT reused as the stationary
    operand -- kT is never materialized.
  * bias algebra: (hsWq)bk^T + bq bk^T are constant along t ->
    softmax-invariant -> dropped exactly.  bq(hsWk)^T varies per t:
    rvec[t] = hs_t . (Wk bq) is folded into the exp bias next to the
    mask.  w~ = 0.125*(Wk bq) comes from one fused DVE
    tensor_tensor_reduce per Wk chunk; rvec accumulates via N=1
    rider matmuls inside scores block 0 (stationary already loaded,
    ~4ns each).
  * V projection runs FIRST (dense 1.7us-per-512KB-chunk PE work that
    tracks the hs DMA stream), so the startup is PE-bound almost
    immediately; W transposes / A / t1 / scores / ctx follow with the
    DMA queue always ahead of the PE.
  * phase 2 (scores s-blocks software-pipelined with ctx blocks,
    rowsum fused via ones-columns of v) is unchanged from v2.

Numerics (numpy simulation of the exact rounding chain): A-trick
rel-to-max-|out| 3.1e-3 vs baseline chain 3.6e-3; gate is 2e-2.
"""

import sys

sys.path.insert(0, "/opt/trn_rl_repo")

from contextlib import ExitStack

import numpy as np

import concourse.bass as bass
import concourse.mybir as mybir
import concourse.tile as tile
from concourse import bacc
from concourse.bass_utils import run_bass_kernel_spmd
from concourse.masks import make_identity

B, S, D = 8, 2048, 1024
NCORES = 8
PD = 128            # partition dim
DK = D // PD        # 8 contraction chunks
SC = S // PD        # 16 sequence chunks
NT = 512            # matmul moving-dim tile (one PSUM bank of fp32)
SBLK = 512          # attention s-block
NBLK = S // SBLK    # 4
VW = D + 4          # v row width incl. ones cols for the fused rowsum
F32 = mybir.dt.float32
F16 = mybir.dt.float16
BF16 = mybir.dt.bfloat16
F32R = mybir.dt.float32r
EXP = mybir.ActivationFunctionType.Exp
MULT = mybir.AluOpType.mult
ADD = mybir.AluOpType.add

BIAS_TTR = True    # w~ via fused tensor_tensor_reduce
BIAS_RIDER = True  # rvec riders + mask2 in scores block 0
_compiled_nc = None


def _build():
    nc = bacc.Bacc(
        "TRN2",
        target_bir_lowering=False,
        debug=False,
        num_devices=NCORES,
        enable_asserts=False,
    )
    hs = nc.dram_tensor("hidden_states", [S, D], F32, kind="ExternalInput").ap()
    mask = nc.dram_tensor("attention_mask", [1, S], F32, kind="ExternalInput").ap()
    Wq = nc.dram_tensor("Wq", [D, D], F32, kind="ExternalInput").ap()
    bq = nc.dram_tensor("bq", [D], F32, kind="ExternalInput").ap()
    Wk = nc.dram_tensor("Wk", [D, D], F32, kind="ExternalInput").ap()
    bk = nc.dram_tensor("bk", [D], F32, kind="ExternalInput").ap()  # noqa: F841  (softmax-invariant, unused)
    Wv = nc.dram_tensor("Wv", [D, D], F32, kind="ExternalInput").ap()
    bv = nc.dram_tensor("bv", [D], F32, kind="ExternalInput").ap()
    out = nc.dram_tensor("context", [S, D], F32, kind="ExternalOutput").ap()

    with tile.TileContext(nc) as tc, ExitStack() as ctx:
        persist = ctx.enter_context(tc.tile_pool(name="persist", bufs=1))
        dramp = ctx.enter_context(tc.tile_pool(name="dram", bufs=1, space="DRAM"))

        hsT = persist.tile([PD, DK, S], F16)     # [e-part, dk, s] = hs^T
        t1T = persist.tile([PD, DK, S], F16)     # [d-part, m, s] = (hs A)^T
        v_sb = persist.tile([PD, SC, VW], BF16)  # [t-part, c, d | ones]

        mask_sb = persist.tile([PD, SC], F32)    # raw mask per t-chunk
        mask2 = persist.tile([PD, SC], F32)      # mask + 0.125*rvec (exp bias)
        wt_sb = persist.tile([PD, DK], F16)      # 0.125 * (Wk @ bq), e-chunked

        with ExitStack() as p1:
            wbig = p1.enter_context(tc.tile_pool(name="wbig", bufs=1))
            WkT = wbig.tile([PD, DK, D], F16)    # [f-part, fc, d] = Wk^T
            wv16 = wbig.tile([PD, DK, D], F16)   # [e-part, ec, f] = Wv

            rowp = p1.enter_context(tc.tile_pool(name="rows", bufs=1))
            bq_raw = rowp.tile([1, D], F32)
            bv_raw = rowp.tile([1, D], F32)
            bq16 = rowp.tile([1, D], F16)
            bv16 = rowp.tile([1, D], F16)
            ones16 = rowp.tile([1, PD], F16)
            mask_row = rowp.tile([SC, PD], F32)
            wt_raw = rowp.tile([PD, DK], F32)
            ttr_out = rowp.tile([PD, D], F32)    # product scratch for the w~ reduce
            bq_row = rowp.tile([PD, D], F16)     # bq broadcast across partitions
            bv_row = rowp.tile([PD, D], F16)     # bv broadcast across partitions
            ident = rowp.tile([PD, PD], F32)
            ident16 = rowp.tile([PD, PD], F16)
            ident_r = rowp.tile([PD, PD], F32R)

            junksp = p1.enter_context(tc.tile_pool(name="junks", bufs=1))

            wchp = p1.enter_context(tc.tile_pool(name="wch", bufs=4))

            ptr = p1.enter_context(tc.tile_pool(name="ptr", bufs=3, space="PSUM"))
            pbias = p1.enter_context(
                tc.tile_pool(name="pbias", bufs=1, space="PSUM")
            )
            junkp = p1.enter_context(
                tc.tile_pool(name="junkp", bufs=1, space="PSUM")
            )
            pp = p1.enter_context(tc.tile_pool(name="pp", bufs=2, space="PSUM"))

            # ---- DMA sequencing.  Single sync queue, in-order; desired
            # arrival order: bq, bv, hs0, wv0-3, hs1..15, wv4-7, wk0-7,
            # wq0-7, mask.  Buffer-rotating pools get their next DMA
            # emitted only after the previous epoch's readers.
            nc.sync.dma_start(out=bq_raw, in_=bq.rearrange("(c p) -> c p", c=1))
            nc.sync.dma_start(out=bv_raw, in_=bv.rearrange("(c p) -> c p", c=1))

            hchunks = {}

            def issue_hs(sc):
                if sc >= SC:
                    return
                hchunk = hsp.tile([PD, D], F32, name="hchunk", tag="hchunk")
                nc.sync.dma_start(out=hchunk, in_=hs[sc * PD : (sc + 1) * PD, :])
                hchunks[sc] = hchunk

            wvst_stage = {}

            def issue_wv(m):
                wvst = wvp.tile([PD, DK, PD], F32, name="wvst", tag="wvst")
                nc.sync.dma_start(
                    out=wvst,
                    in_=Wv.rearrange("(dk p) n -> p dk n", p=PD)[
                        :, :, m * PD : (m + 1) * PD
                    ],
                )
                wvst_stage[m] = wvst

            def cast_wv(m):
                nc.vector.tensor_copy(
                    out=wv16[:, :, m * PD : (m + 1) * PD], in_=wvst_stage.pop(m)
                )

            wch_stage = {}

            def issue_wch(W, c):
                wch = wchp.tile([PD, D], F32, name="wch", tag="wch")
                nc.sync.dma_start(out=wch, in_=W[c * PD : (c + 1) * PD, :])
                wch_stage[(id(W), c)] = wch

            with ExitStack() as p2:
                hsp = p2.enter_context(tc.tile_pool(name="hsload", bufs=3))
                h16p = p2.enter_context(tc.tile_pool(name="hs16", bufs=2))
                wvp = p2.enter_context(tc.tile_pool(name="wvst", bufs=2))

                issue_hs(0)
                issue_wv(0)
                issue_wv(1)
                issue_hs(1)
                issue_hs(2)

                # ---- PE warmup: junk matmuls on a memset tile so the HAM
                # clock gate opens before the real work arrives.
                jnk16 = junksp.tile([PD, 256], F16, name="jnk16", tag="jnk16")
                nc.vector.memset(jnk16, 0.25)
                make_identity(nc, ident)
                nc.vector.tensor_copy(out=ident16, in_=ident)
                nc.vector.tensor_copy(out=ident_r, in_=ident)
                nc.vector.memset(ones16, 1.0)
                nc.vector.memset(v_sb[:, :, D:VW], 1.0)
                warm_ps = junkp.tile([PD, NT], F32, name="warm_ps", tag="warm_ps")
                for _ in range(18):
                    nc.tensor.matmul(
                        out=warm_ps[:, 0:256],
                        lhsT=jnk16[:, 0:PD],
                        rhs=jnk16,
                        start=True,
                        stop=True,
                    )

                cast_wv(0)
                issue_wv(2)
                cast_wv(1)
                issue_wv(3)

                # bq/bv broadcast rows via rank-1 matmuls (ones x bias-row)
                nc.scalar.mul(out=bq16, in_=bq_raw, mul=0.125)  # fold exp scale into w~
                nc.scalar.copy(out=bv16, in_=bv_raw)
                for src, dst in ((bq16, bq_row), (bv16, bv_row)):
                    for dt in range(D // NT):
                        pbv = pbias.tile([PD, NT], F32, name="pb", tag="pbias")
                        nc.tensor.matmul(
                            out=pbv,
                            lhsT=ones16,
                            rhs=src[:, dt * NT : (dt + 1) * NT],
                            start=True,
                            stop=True,
                        )
                        nc.vector.tensor_copy(
                            out=dst[:, dt * NT : (dt + 1) * NT], in_=pbv
                        )

                cast_wv(2)
                cast_wv(3)

                # ---- transpose one hs chunk into hsT via PE matmuls ----
                def transpose_chunk(h16, dst, sc):
                    for half in range(2):
                        pst = ptr.tile([PD, 4, PD], F32)
                        for j in range(4):
                            dk = half * 4 + j
                            nc.tensor.matmul(
                                out=pst[:, j, :],
                                lhsT=h16[:, dk * PD : (dk + 1) * PD],
                                rhs=ident16,
                                start=True,
                                stop=True,
                            )
                        nc.vector.tensor_copy(
                            out=dst[
                                :, half * 4 : (half + 1) * 4, sc * PD : (sc + 1) * PD
                            ],
                            in_=pst,
                        )

                def vblock(dt, c):
                    ps = pp.tile([PD, NT], F32)
                    for dk in range(DK):
                        nc.tensor.matmul(
                            out=ps,
                            lhsT=hsT[:, dk, c * PD : (c + 1) * PD],
                            rhs=wv16[:, dk, dt * NT : (dt + 1) * NT],
                            start=(dk == 0),
                            stop=(dk == DK - 1),
                        )
                    nc.vector.tensor_add(
                        out=v_sb[:, c, dt * NT : (dt + 1) * NT],
                        in0=ps,
                        in1=bv_row[:, dt * NT : (dt + 1) * NT],
                    )

                # ---- main hs loop: transpose + V projection dt=0 ----
                for c in range(SC):
                    h16 = h16p.tile([PD, D], F16, name="h16", tag="h16")
                    nc.scalar.copy(out=h16, in_=hchunks.pop(c))
                    issue_hs(c + 3)
                    if c == 13:
                        issue_wv(4)
                        issue_wv(5)
                    transpose_chunk(h16, hsT, c)
                    vblock(0, c)

                cast_wv(4)
                issue_wv(6)
                cast_wv(5)
                issue_wv(7)
                cast_wv(6)
                cast_wv(7)
                issue_wch(Wk, 0)
                issue_wch(Wk, 1)
                issue_wch(Wk, 2)

                # ---- V projection dt=1 (pure PE filler while Wk/Wq load)
                for c in range(SC):
                    vblock(1, c)
                    if c < DK:
                        # w~ = 0.125*(Wk bq) per chunk, fused mult+reduce
                        if BIAS_TTR:
                            wch = wch_stage[(id(Wk), c)]
                            nc.vector.tensor_mul(
                            out=ttr_out, in0=wch.bitcast(F32), in1=bq_row
                        )
                            nc.vector.tensor_reduce(
                                out=wt_raw[:, c : c + 1],
                                in_=ttr_out,
                                axis=mybir.AxisListType.X,
                                op=ADD,
                            )
                        w16c = w16p.tile([PD, D], F16, name="w16c", tag="w16c")
                        nc.scalar.copy(out=w16c, in_=wch_stage.pop((id(Wk), c)))
                        transpose_chunk(w16c, WkT, c)
                        if c < 5:
                            issue_wch(Wk, c + 3)
                        elif c == 5:
                            issue_wch(Wq, 0)
                        elif c == 6:
                            issue_wch(Wq, 1)
                        elif c == 7:
                            issue_wch(Wq, 2)
                            if BIAS_TTR:
                                nc.vector.tensor_copy(out=wt_sb, in_=wt_raw)

            # staging pools for hs/wv closed; their SBUF space is reused
            # for WqT and A (never live at the same time as the staging).
            wbig2 = p1.enter_context(tc.tile_pool(name="wbig2", bufs=1))
            WqT = wbig2.tile([PD, DK, D], F16)   # [f-part, fc, e] = Wq^T
            A_sb = wbig2.tile([PD, DK, D], F16)  # [e-part, ec, d] = Wq Wk^T

            # ---- A = Wq Wk^T, chunk ec at a time as Wq chunks land ----
            def a_block(ec, half):
                ps = pp.tile([PD, NT], F32)
                for fc in range(DK):
                    nc.tensor.matmul(
                        out=ps,
                        lhsT=WqT[:, fc, ec * PD : (ec + 1) * PD],
                        rhs=WkT[:, fc, half * NT : (half + 1) * NT],
                        start=(fc == 0),
                        stop=(fc == DK - 1),
                    )
                nc.vector.tensor_copy(
                    out=A_sb[:, ec, half * NT : (half + 1) * NT], in_=ps
                )

            for ec in range(DK):
                w16c = w16p.tile([PD, D], F16, name="w16c", tag="w16c")
                nc.scalar.copy(out=w16c, in_=wch_stage.pop((id(Wq), ec)))
                if ec < 5:
                    issue_wch(Wq, ec + 3)
                elif ec == 5:
                    nc.sync.dma_start(
                        out=mask_row,
                        in_=mask[0, :].rearrange("(c p) -> c p", c=SC),
                    )
                transpose_chunk(w16c, WqT, ec)
                a_block(ec, 0)
                a_block(ec, 1)
                if ec == 6:
                    pbm = pbias.tile([PD, NT], F32, name="pb", tag="pbias")
                    nc.tensor.transpose(
                        out=pbm[:, 0:SC], in_=mask_row, identity=ident[0:SC, 0:SC]
                    )
                    nc.vector.tensor_copy(out=mask_sb, in_=pbm[:, 0:SC])

            # ---- t1T = A^T hs^T  (replaces the old Q projection) ----
            for tb in range(4):
                for m in range(DK):
                    ps = pp.tile([PD, NT], F32)
                    for ec in range(DK):
                        nc.tensor.matmul(
                            out=ps,
                            lhsT=A_sb[:, ec, m * PD : (m + 1) * PD],
                            rhs=hsT[:, ec, tb * NT : (tb + 1) * NT],
                            start=(ec == 0),
                            stop=(ec == DK - 1),
                        )
                    nc.vector.tensor_copy(
                        out=t1T[:, m, tb * NT : (tb + 1) * NT], in_=ps
                    )

            # junk-warmup drain (kept alive for DCE; emitted late so its
            # deps never gate the startup queues)
            warm_sb = junksp.tile([PD, PD], F32, name="warm_sb", tag="warm_sb")
            nc.vector.tensor_copy(out=warm_sb, in_=warm_ps[:, 0:PD])
            warm_dram = dramp.tile([PD, PD], F32, name="warm_dram", tag="warm_dram")
            nc.sync.dma_start(out=warm_dram[:, :], in_=warm_sb)

        # ---- phase 2: attention, software-pipelined S0 S1 C0 S2 C1 ...
        with (
            tc.tile_pool(name="expp", bufs=3) as epool,
            tc.tile_pool(name="outp", bufs=2) as opool,
            tc.tile_pool(name="rcp", bufs=4) as rpool,
            tc.tile_pool(name="psc", bufs=2, space="PSUM") as psc,
            tc.tile_pool(name="pca", bufs=2, space="PSUM") as pca,
            tc.tile_pool(name="pcb", bufs=2, space="PSUM") as pcb,
            tc.tile_pool(name="pcr", bufs=1, space="PSUM") as pcr,
            tc.tile_pool(name="prvp", bufs=1, space="PSUM") as prvp,
        ):
            prv = prvp.tile([PD, SC], F32, name="prv", tag="prv")

            def scores_block(sb):
                exp_sb = epool.tile(
                    [PD, SC, SBLK], BF16, name="exp_sb", tag="exp_sb"
                )
                for tcn in range(SC):
                    ps = psc.tile([PD, SBLK], F32)
                    for dk in range(DK):
                        nc.tensor.matmul(
                            out=ps,
                            lhsT=hsT[:, dk, tcn * PD : (tcn + 1) * PD],
                            rhs=t1T[:, dk, sb * SBLK : (sb + 1) * SBLK],
                            start=(dk == 0),
                            stop=(dk == DK - 1),
                        )
                        if sb == 0 and BIAS_RIDER:
                            # rvec rider: same stationary, N=1, ~4ns
                            nc.tensor.matmul(
                                out=prv[:, tcn : tcn + 1],
                                lhsT=hsT[:, dk, tcn * PD : (tcn + 1) * PD],
                                rhs=wt_sb[:, dk : dk + 1],
                                start=(dk == 0),
                                stop=(dk == DK - 1),
                            )
                    if sb == 0 and BIAS_RIDER:
                        nc.vector.tensor_add(
                            out=mask2[:, tcn : tcn + 1],
                            in0=mask_sb[:, tcn : tcn + 1],
                            in1=prv[:, tcn : tcn + 1],
                        )
                    nc.scalar.activation(
                        out=exp_sb[:, tcn, :],
                        in_=ps,
                        func=EXP,
                        scale=0.125,
                        bias=(mask2 if BIAS_RIDER else mask_sb)[:, tcn : tcn + 1],
                    )
                return exp_sb

            def context_block(sb, exp_sb):
                for ss in range(SBLK // PD):
                    pa = pca.tile([PD, NT], F32)
                    pb = pcb.tile([PD, NT], F32)
                    pr = pcr.tile([PD, NT], F32)
                    for tcn in range(SC):
                        st, sp = (tcn == 0), (tcn == SC - 1)
                        e_sl = exp_sb[:, tcn, ss * PD : (ss + 1) * PD]
                        nc.tensor.matmul(
                            out=pa, lhsT=e_sl, rhs=v_sb[:, tcn, 0:NT],
                            start=st, stop=sp,
                        )
                        nc.tensor.matmul(
                            out=pb, lhsT=e_sl, rhs=v_sb[:, tcn, NT : 2 * NT],
                            start=st, stop=sp,
                        )
                        nc.tensor.matmul(
                            out=pr[:, 0:4], lhsT=e_sl, rhs=v_sb[:, tcn, D:VW],
                            start=st, stop=sp,
                        )
                    recip = rpool.tile([PD, 1], F32, name="recip_t", tag="recip_t")
                    nc.vector.reciprocal(out=recip, in_=pr[:, 0:1])
                    ostage = opool.tile([PD, D], F32)
                    nc.vector.tensor_scalar_mul(
                        out=ostage[:, 0:NT], in0=pa, scalar1=recip
                    )
                    nc.vector.tensor_scalar_mul(
                        out=ostage[:, NT : 2 * NT], in0=pb, scalar1=recip
                    )
                    row = sb * SBLK + ss * PD
                    nc.sync.dma_start(out=out[row : row + PD, :], in_=ostage)

            pending = []
            for sb in range(NBLK):
                e = scores_block(sb)
                pending.append((sb, e))
                if sb >= 1:
                    context_block(*pending.pop(0))
            while pending:
                context_block(*pending.pop(0))

    nc.compile()
    return nc


def _get_compiled():
    global _compiled_nc
    if _compiled_nc is None:
        _compiled_nc = _build()
    return _compiled_nc


def _run(inputs, **kwargs):
    hs = np.asarray(inputs["hidden_states"], dtype=np.float32)
    mask = np.asarray(inputs["attention_mask"], dtype=np.float32)
    ws = {
        k: np.ascontiguousarray(np.asarray(inputs[k], dtype=np.float32))
        for k in ("Wq", "bq", "Wk", "bk", "Wv", "bv")
    }
    nc = _get_compiled()
    in_maps = [
        {
            "hidden_states": np.ascontiguousarray(hs[i]),
            "attention_mask": np.ascontiguousarray(mask[i]),
            **ws,
        }
        for i in range(NCORES)
    ]
    r = run_bass_kernel_spmd(nc, in_maps, list(range(NCORES)), **kwargs)
    out = np.stack([r.results[i]["context"] for i in range(NCORES)], axis=0)
    return out, r


def kernel(**inputs) -> np.ndarray:
    out, _ = _run(inputs)
    return out


if __name__ == "__main__":
    rng = np.random.default_rng(0)
    scale = 1.0 / np.sqrt(D)
    inputs = {
        "hidden_states": rng.standard_normal((B, S, D)).astype(np.float32),
        "attention_mask": np.zeros((B, 1, S), np.float32),
        "Wq": (rng.standard_normal((D, D)) * scale).astype(np.float32),
        "bq": (rng.standard_normal(D) * 0.05).astype(np.float32),
        "Wk": (rng.standard_normal((D, D)) * scale).astype(np.float32),
        "bk": (rng.standard_normal(D) * 0.05).astype(np.float32),
        "Wv": (rng.standard_normal((D, D)) * scale).astype(np.float32),
        "bv": (rng.standard_normal(D) * 0.05).astype(np.float32),
    }
    got = kernel(**inputs)

    hs64 = inputs["hidden_states"].astype(np.float64)
    q = hs64 @ inputs["Wq"].astype(np.float64) + inputs["bq"].astype(np.float64)
    k = hs64 @ inputs["Wk"].astype(np.float64) + inputs["bk"].astype(np.float64)
    v = hs64 @ inputs["Wv"].astype(np.float64) + inputs["bv"].astype(np.float64)
    sc = np.einsum("bsd,btd->bst", q, k) / 8.0
    sc += inputs["attention_mask"].astype(np.float64)
    sc -= sc.max(axis=-1, keepdims=True)
    p = np.exp(sc)
    p /= p.sum(axis=-1, keepdims=True)
    ref = np.einsum("bst,btd->bsd", p, v)
    err = np.abs(got.astype(np.float64) - ref)
    print(
        f"absmax={err.max():.3e} rel_vs_scale={err.max() / np.abs(ref).max():.3e} "
        f"rms_rel={np.sqrt((err**2).mean()) / np.sqrt((ref**2).mean()):.3e}"
    )


# revision 17
# speedup vs baseline: 1.0100x; 1.0100x over previous
"""Trainium2 Bass kernel for CustomBertSelfAttention (no head split).

reference:
    q = hs @ Wq + bq; k = hs @ Wk + bk; v = hs @ Wv + bv        # [B,S,D]
    scores = (q @ k^T) / sqrt(64) + mask                         # [B,S,S]
    probs  = softmax(scores, -1)
    out    = probs @ v                                           # [B,S,D]

B=8, S=2048, D=1024.  Sharding: data-parallel over batch, one batch
element per NeuronCore (8 cores), no collectives.

v3 plan -- algebraic restructure on top of the all-16-bit v2:
  * scores = q k^T = hs (Wq Wk^T) hs^T + per-t/per-s bias terms.
    Precompute A = Wq Wk^T once per core (1024^3 matmul, 27.6us)
    and drop the entire K projection (55.3us): scores^T[t,s] =
    hsT^T_chunks @ (A^T hsT) with hsT reused as the stationary
    operand -- kT is never materialized.
  * bias algebra: (hsWq)bk^T + bq bk^T are constant along t ->
    softmax-invariant -> dropped exactly.  bq(hsWk)^T varies per t:
    rvec[t] = hs_t . (Wk bq) is folded into the exp bias next to the
    mask.  w~ = 0.125*(Wk bq) comes from one fused DVE
    tensor_tensor_reduce per Wk chunk; rvec accumulates via N=1
    rider matmuls inside scores block 0 (stationary already loaded,
    ~4ns each).
  * V projection runs FIRST (dense 1.7us-per-512KB-chunk PE work that
    tracks the hs DMA stream), so the startup is PE-bound almost
    immediately; W transposes / A / t1 / scores / ctx follow with the
    DMA queue always ahead of the PE.
  * phase 2 (scores s-blocks software-pipelined with ctx blocks,
    rowsum fused via ones-columns of v) is unchanged from v2.

Numerics (numpy simulation of the exact rounding chain): A-trick
rel-to-max-|out| 3.1e-3 vs baseline chain 3.6e-3; gate is 2e-2.
"""

import sys

sys.path.insert(0, "/opt/trn_rl_repo")

from contextlib import ExitStack

import numpy as np

import concourse.bass as bass
import concourse.mybir as mybir
import concourse.tile as tile
from concourse import bacc
from concourse.bass_utils import run_bass_kernel_spmd
from concourse.masks import make_identity

B, S, D = 8, 2048, 1024
NCORES = 8
PD = 128            # partition dim
DK = D // PD        # 8 contraction chunks
SC = S // PD        # 16 sequence chunks
NT = 512            # matmul moving-dim tile (one PSUM bank of fp32)
SBLK = 512          # attention s-block
NBLK = S // SBLK    # 4
VW = D + 4          # v row width incl. ones cols for the fused rowsum
F32 = mybir.dt.float32
F16 = mybir.dt.float16
BF16 = mybir.dt.bfloat16
EXP = mybir.ActivationFunctionType.Exp
MULT = mybir.AluOpType.mult
ADD = mybir.AluOpType.add

_compiled_nc = None


def _build():
    nc = bacc.Bacc(
        "TRN2",
        target_bir_lowering=False,
        debug=False,
        num_devices=NCORES,
        enable_asserts=False,
    )
    hs = nc.dram_tensor("hidden_states", [S, D], F32, kind="ExternalInput").ap()
    mask = nc.dram_tensor("attention_mask", [1, S], F32, kind="ExternalInput").ap()
    Wq = nc.dram_tensor("Wq", [D, D], F32, kind="ExternalInput").ap()
    bq = nc.dram_tensor("bq", [D], F32, kind="ExternalInput").ap()
    Wk = nc.dram_tensor("Wk", [D, D], F32, kind="ExternalInput").ap()
    bk = nc.dram_tensor("bk", [D], F32, kind="ExternalInput").ap()  # noqa: F841  (softmax-invariant, unused)
    Wv = nc.dram_tensor("Wv", [D, D], F32, kind="ExternalInput").ap()
    bv = nc.dram_tensor("bv", [D], F32, kind="ExternalInput").ap()
    out = nc.dram_tensor("context", [S, D], F32, kind="ExternalOutput").ap()

    with tile.TileContext(nc) as tc, ExitStack() as ctx:
        persist = ctx.enter_context(tc.tile_pool(name="persist", bufs=1))
        dramp = ctx.enter_context(tc.tile_pool(name="dram", bufs=1, space="DRAM"))

        hsT = persist.tile([PD, DK, S], F16)     # [e-part, dk, s] = hs^T
        t1T = persist.tile([PD, DK, S], F16)     # [d-part, m, s] = (hs A)^T
        v_sb = persist.tile([PD, SC, VW], BF16)  # [t-part, c, d | ones]

        mask_sb = persist.tile([PD, SC], F32)    # raw mask per t-chunk
        mask2 = persist.tile([PD, SC], F32)      # mask + 0.125*rvec (exp bias)
        wt_sb = persist.tile([PD, DK], F16)      # 0.125 * (Wk @ bq), e-chunked

        with ExitStack() as p1:
            wbig = p1.enter_context(tc.tile_pool(name="wbig", bufs=1))
            WkT = wbig.tile([PD, DK, D], F16)    # [f-part, fc, d] = Wk^T
            wv16 = wbig.tile([PD, DK, D], F16)   # [e-part, ec, f] = Wv

            rowp = p1.enter_context(tc.tile_pool(name="rows", bufs=1))
            bq_raw = rowp.tile([1, D], F32)
            bv_raw = rowp.tile([1, D], F32)
            bq16 = rowp.tile([1, D], F16)
            bv16 = rowp.tile([1, D], F16)
            ones16 = rowp.tile([1, PD], F16)
            mask_row = rowp.tile([SC, PD], F32)
            wt_raw = rowp.tile([PD, DK], F32)
            ttr_out = rowp.tile([PD, D], F32)    # product scratch for the w~ reduce
            bq_row = rowp.tile([PD, D], F16)     # bq broadcast across partitions
            bv_row = rowp.tile([PD, D], F16)     # bv broadcast across partitions
            ident = rowp.tile([PD, PD], F32)
            ident16 = rowp.tile([PD, PD], F16)

            junksp = p1.enter_context(tc.tile_pool(name="junks", bufs=1))

            wchp = p1.enter_context(tc.tile_pool(name="wch", bufs=3))
            w16p = p1.enter_context(tc.tile_pool(name="w16c", bufs=2))

            ptr = p1.enter_context(tc.tile_pool(name="ptr", bufs=3, space="PSUM"))
            pbias = p1.enter_context(
                tc.tile_pool(name="pbias", bufs=1, space="PSUM")
            )
            junkp = p1.enter_context(
                tc.tile_pool(name="junkp", bufs=1, space="PSUM")
            )
            pp = p1.enter_context(tc.tile_pool(name="pp", bufs=2, space="PSUM"))

            # ---- DMA sequencing.  Single sync queue, in-order; desired
            # arrival order: bq, bv, hs0, wv0-3, hs1..15, wv4-7, wk0-7,
            # wq0-7, mask.  Buffer-rotating pools get their next DMA
            # emitted only after the previous epoch's readers.
            nc.sync.dma_start(out=bq_raw, in_=bq.rearrange("(c p) -> c p", c=1))
            nc.sync.dma_start(out=bv_raw, in_=bv.rearrange("(c p) -> c p", c=1))

            hchunks = {}

            def issue_hs(sc):
                if sc >= SC:
                    return
                hchunk = hsp.tile([PD, D], F32, name="hchunk", tag="hchunk")
                nc.sync.dma_start(out=hchunk, in_=hs[sc * PD : (sc + 1) * PD, :])
                hchunks[sc] = hchunk

            wvst_stage = {}

            def issue_wv(m):
                wvst = wvp.tile([PD, DK, PD], F32, name="wvst", tag="wvst")
                nc.sync.dma_start(
                    out=wvst,
                    in_=Wv.rearrange("(dk p) n -> p dk n", p=PD)[
                        :, :, m * PD : (m + 1) * PD
                    ],
                )
                wvst_stage[m] = wvst

            def cast_wv(m):
                nc.vector.tensor_copy(
                    out=wv16[:, :, m * PD : (m + 1) * PD], in_=wvst_stage.pop(m)
                )

            wch_stage = {}

            def issue_wch(W, c):
                wch = wchp.tile([PD, D], F32, name="wch", tag="wch")
                nc.sync.dma_start(out=wch, in_=W[c * PD : (c + 1) * PD, :])
                wch_stage[(id(W), c)] = wch

            with ExitStack() as p2:
                hsp = p2.enter_context(tc.tile_pool(name="hsload", bufs=3))
                h16p = p2.enter_context(tc.tile_pool(name="hs16", bufs=2))
                wvp = p2.enter_context(tc.tile_pool(name="wvst", bufs=2))

                issue_hs(0)
                issue_wv(0)
                issue_wv(1)
                issue_hs(1)
                issue_hs(2)

                # ---- PE warmup: junk matmuls on a memset tile so the HAM
                # clock gate opens before the real work arrives.
                jnk16 = junksp.tile([PD, 256], F16, name="jnk16", tag="jnk16")
                nc.vector.memset(jnk16, 0.25)
                make_identity(nc, ident)
                nc.vector.tensor_copy(out=ident16, in_=ident)
                nc.vector.memset(ones16, 1.0)
                nc.vector.memset(v_sb[:, :, D:VW], 1.0)
                warm_ps = junkp.tile([PD, NT], F32, name="warm_ps", tag="warm_ps")
                for _ in range(18):
                    nc.tensor.matmul(
                        out=warm_ps[:, 0:256],
                        lhsT=jnk16[:, 0:PD],
                        rhs=jnk16,
                        start=True,
                        stop=True,
                    )

                cast_wv(0)
                issue_wv(2)
                cast_wv(1)
                issue_wv(3)

                # bq/bv broadcast rows via rank-1 matmuls (ones x bias-row)
                nc.scalar.mul(out=bq16, in_=bq_raw, mul=0.125)  # fold exp scale into w~
                nc.scalar.copy(out=bv16, in_=bv_raw)
                for src, dst in ((bq16, bq_row), (bv16, bv_row)):
                    for dt in range(D // NT):
                        pbv = pbias.tile([PD, NT], F32, name="pb", tag="pbias")
                        nc.tensor.matmul(
                            out=pbv,
                            lhsT=ones16,
                            rhs=src[:, dt * NT : (dt + 1) * NT],
                            start=True,
                            stop=True,
                        )
                        nc.vector.tensor_copy(
                            out=dst[:, dt * NT : (dt + 1) * NT], in_=pbv
                        )

                cast_wv(2)
                cast_wv(3)

                # ---- transpose one hs chunk into hsT via PE matmuls ----
                def transpose_chunk(h16, dst, sc):
                    for half in range(2):
                        pst = ptr.tile([PD, 4, PD], F32)
                        for j in range(4):
                            dk = half * 4 + j
                            nc.tensor.matmul(
                                out=pst[:, j, :],
                                lhsT=h16[:, dk * PD : (dk + 1) * PD],
                                rhs=ident16,
                                start=True,
                                stop=True,
                            )
                        nc.vector.tensor_copy(
                            out=dst[
                                :, half * 4 : (half + 1) * 4, sc * PD : (sc + 1) * PD
                            ],
                            in_=pst,
                        )

                def vblock(dt, c):
                    ps = pp.tile([PD, NT], F32)
                    for dk in range(DK):
                        nc.tensor.matmul(
                            out=ps,
                            lhsT=hsT[:, dk, c * PD : (c + 1) * PD],
                            rhs=wv16[:, dk, dt * NT : (dt + 1) * NT],
                            start=(dk == 0),
                            stop=(dk == DK - 1),
                        )
                    nc.vector.tensor_add(
                        out=v_sb[:, c, dt * NT : (dt + 1) * NT],
                        in0=ps,
                        in1=bv_row[:, dt * NT : (dt + 1) * NT],
                    )

                # ---- main hs loop: transpose + V projection dt=0 ----
                for c in range(SC):
                    h16 = h16p.tile([PD, D], F16, name="h16", tag="h16")
                    nc.scalar.copy(out=h16, in_=hchunks.pop(c))
                    issue_hs(c + 3)
                    if c == 13:
                        issue_wv(4)
                        issue_wv(5)
                    transpose_chunk(h16, hsT, c)
                    vblock(0, c)

                cast_wv(4)
                issue_wv(6)
                cast_wv(5)
                issue_wv(7)
                cast_wv(6)
                cast_wv(7)
                issue_wch(Wk, 0)
                issue_wch(Wk, 1)
                issue_wch(Wk, 2)

                # ---- V projection dt=1 (pure PE filler while Wk/Wq load)
                for c in range(SC):
                    vblock(1, c)
                    if c < DK:
                        # w~ = 0.125*(Wk bq) per chunk, fused mult+reduce
                        wch = wch_stage[(id(Wk), c)]
                        nc.vector.tensor_mul(out=ttr_out, in0=wch, in1=bq_row)
                        nc.vector.tensor_reduce(
                            out=wt_raw[:, c : c + 1],
                            in_=ttr_out,
                            axis=mybir.AxisListType.X,
                            op=ADD,
                        )
                        w16c = w16p.tile([PD, D], F16, name="w16c", tag="w16c")
                        nc.scalar.copy(out=w16c, in_=wch_stage.pop((id(Wk), c)))
                        transpose_chunk(w16c, WkT, c)
                        if c < 5:
                            issue_wch(Wk, c + 3)
                        elif c == 5:
                            issue_wch(Wq, 0)
                        elif c == 6:
                            issue_wch(Wq, 1)
                        elif c == 7:
                            issue_wch(Wq, 2)
                            nc.vector.tensor_copy(out=wt_sb, in_=wt_raw)

            # staging pools for hs/wv closed; their SBUF space is reused
            # for WqT and A (never live at the same time as the staging).
            wbig2 = p1.enter_context(tc.tile_pool(name="wbig2", bufs=1))
            WqT = wbig2.tile([PD, DK, D], F16)   # [f-part, fc, e] = Wq^T
            A_sb = wbig2.tile([PD, DK, D], F16)  # [e-part, ec, d] = Wq Wk^T

            # ---- A = Wq Wk^T, chunk ec at a time as Wq chunks land ----
            def a_block(ec, half):
                ps = pp.tile([PD, NT], F32)
                for fc in range(DK):
                    nc.tensor.matmul(
                        out=ps,
                        lhsT=WqT[:, fc, ec * PD : (ec + 1) * PD],
                        rhs=WkT[:, fc, half * NT : (half + 1) * NT],
                        start=(fc == 0),
                        stop=(fc == DK - 1),
                    )
                nc.vector.tensor_copy(
                    out=A_sb[:, ec, half * NT : (half + 1) * NT], in_=ps
                )

            for ec in range(DK):
                w16c = w16p.tile([PD, D], F16, name="w16c", tag="w16c")
                nc.scalar.copy(out=w16c, in_=wch_stage.pop((id(Wq), ec)))
                if ec < 5:
                    issue_wch(Wq, ec + 3)
                elif ec == 5:
                    nc.sync.dma_start(
                        out=mask_row,
                        in_=mask[0, :].rearrange("(c p) -> c p", c=SC),
                    )
                transpose_chunk(w16c, WqT, ec)
                a_block(ec, 0)
                a_block(ec, 1)
                if ec == 6:
                    pbm = pbias.tile([PD, NT], F32, name="pb", tag="pbias")
                    nc.tensor.transpose(
                        out=pbm[:, 0:SC], in_=mask_row, identity=ident[0:SC, 0:SC]
                    )
                    nc.vector.tensor_copy(out=mask_sb, in_=pbm[:, 0:SC])

            # ---- t1T = A^T hs^T  (replaces the old Q projection) ----
            for tb in range(4):
                for m in range(DK):
                    ps = pp.tile([PD, NT], F32)
                    for ec in range(DK):
                        nc.tensor.matmul(
                            out=ps,
                            lhsT=A_sb[:, ec, m * PD : (m + 1) * PD],
                            rhs=hsT[:, ec, tb * NT : (tb + 1) * NT],
                            start=(ec == 0),
                            stop=(ec == DK - 1),
                        )
                    nc.vector.tensor_copy(
                        out=t1T[:, m, tb * NT : (tb + 1) * NT], in_=ps
                    )

            # junk-warmup drain (kept alive for DCE; emitted late so its
            # deps never gate the startup queues)
            warm_sb = junksp.tile([PD, PD], F32, name="warm_sb", tag="warm_sb")
            nc.vector.tensor_copy(out=warm_sb, in_=warm_ps[:, 0:PD])
            warm_dram = dramp.tile([PD, PD], F32, name="warm_dram", tag="warm_dram")
            nc.sync.dma_start(out=warm_dram[:, :], in_=warm_sb)

        # ---- phase 2: attention, software-pipelined S0 S1 C0 S2 C1 ...
        with (
            tc.tile_pool(name="expp", bufs=3) as epool,
            tc.tile_pool(name="outp", bufs=2) as opool,
            tc.tile_pool(name="rcp", bufs=4) as rpool,
            tc.tile_pool(name="psc", bufs=2, space="PSUM") as psc,
            tc.tile_pool(name="pca", bufs=2, space="PSUM") as pca,
            tc.tile_pool(name="pcb", bufs=2, space="PSUM") as pcb,
            tc.tile_pool(name="pcr", bufs=1, space="PSUM") as pcr,
            tc.tile_pool(name="prvp", bufs=1, space="PSUM") as prvp,
        ):
            prv = prvp.tile([PD, SC], F32, name="prv", tag="prv")

            def scores_block(sb):
                exp_sb = epool.tile(
                    [PD, SC, SBLK], BF16, name="exp_sb", tag="exp_sb"
                )
                for tcn in range(SC):
                    ps = psc.tile([PD, SBLK], F32)
                    for dk in range(DK):
                        nc.tensor.matmul(
                            out=ps,
                            lhsT=hsT[:, dk, tcn * PD : (tcn + 1) * PD],
                            rhs=t1T[:, dk, sb * SBLK : (sb + 1) * SBLK],
                            start=(dk == 0),
                            stop=(dk == DK - 1),
                        )
                        if sb == 0:
                            # rvec rider: same stationary, N=1, ~4ns
                            nc.tensor.matmul(
                                out=prv[:, tcn : tcn + 1],
                                lhsT=hsT[:, dk, tcn * PD : (tcn + 1) * PD],
                                rhs=wt_sb[:, dk : dk + 1],
                                start=(dk == 0),
                                stop=(dk == DK - 1),
                            )
                    if sb == 0:
                        nc.vector.tensor_add(
                            out=mask2[:, tcn : tcn + 1],
                            in0=mask_sb[:, tcn : tcn + 1],
                            in1=prv[:, tcn : tcn + 1],
                        )
                    nc.scalar.activation(
                        out=exp_sb[:, tcn, :],
                        in_=ps,
                        func=EXP,
                        scale=0.125,
                        bias=mask2[:, tcn : tcn + 1],
                    )
                return exp_sb

            def context_block(sb, exp_sb):
                for ss in range(SBLK // PD):
                    pa = pca.tile([PD, NT], F32)
                    pb = pcb.tile([PD, NT], F32)
                    pr = pcr.tile([PD, NT], F32)
                    for tcn in range(SC):
                        st, sp = (tcn == 0), (tcn == SC - 1)
                        e_sl = exp_sb[:, tcn, ss * PD : (ss + 1) * PD]
                        nc.tensor.matmul(
                            out=pa, lhsT=e_sl, rhs=v_sb[:, tcn, 0:NT],
                            start=st, stop=sp,
                        )
                        nc.tensor.matmul(
                            out=pb, lhsT=e_sl, rhs=v_sb[:, tcn, NT : 2 * NT],
                            start=st, stop=sp,
                        )
                        nc.tensor.matmul(
                            out=pr[:, 0:4], lhsT=e_sl, rhs=v_sb[:, tcn, D:VW],
                            start=st, stop=sp,
                        )
                    recip = rpool.tile([PD, 1], F32, name="recip_t", tag="recip_t")
                    nc.vector.reciprocal(out=recip, in_=pr[:, 0:1])
                    ostage = opool.tile([PD, D], F32)
                    nc.vector.tensor_scalar_mul(
                        out=ostage[:, 0:NT], in0=pa, scalar1=recip
                    )
                    nc.vector.tensor_scalar_mul(
                        out=ostage[:, NT : 2 * NT], in0=pb, scalar1=recip
                    )
                    row = sb * SBLK + ss * PD
                    nc.sync.dma_start(out=out[row : row + PD, :], in_=ostage)

            pending = []
            for sb in range(NBLK):
                e = scores_block(sb)
                pending.append((sb, e))
                if sb >= 1:
                    context_block(*pending.pop(0))
            while pending:
                context_block(*pending.pop(0))

    nc.compile()
    return nc


def _get_compiled():
    global _compiled_nc
    if _compiled_nc is None:
        _compiled_nc = _build()
    return _compiled_nc


def _run(inputs, **kwargs):
    hs = np.asarray(inputs["hidden_states"], dtype=np.float32)
    mask = np.asarray(inputs["attention_mask"], dtype=np.float32)
    ws = {
        k: np.ascontiguousarray(np.asarray(inputs[k], dtype=np.float32))
        for k in ("Wq", "bq", "Wk", "bk", "Wv", "bv")
    }
    nc = _get_compiled()
    in_maps = [
        {
            "hidden_states": np.ascontiguousarray(hs[i]),
            "attention_mask": np.ascontiguousarray(mask[i]),
            **ws,
        }
        for i in range(NCORES)
    ]
    r = run_bass_kernel_spmd(nc, in_maps, list(range(NCORES)), **kwargs)
    out = np.stack([r.results[i]["context"] for i in range(NCORES)], axis=0)
    return out, r


def kernel(**inputs) -> np.ndarray:
    out, _ = _run(inputs)
    return out


if __name__ == "__main__":
    rng = np.random.default_rng(0)
    scale = 1.0 / np.sqrt(D)
    inputs = {
        "hidden_states": rng.standard_normal((B, S, D)).astype(np.float32),
        "attention_mask": np.zeros((B, 1, S), np.float32),
        "Wq": (rng.standard_normal((D, D)) * scale).astype(np.float32),
        "bq": (rng.standard_normal(D) * 0.05).astype(np.float32),
        "Wk": (rng.standard_normal((D, D)) * scale).astype(np.float32),
        "bk": (rng.standard_normal(D) * 0.05).astype(np.float32),
        "Wv": (rng.standard_normal((D, D)) * scale).astype(np.float32),
        "bv": (rng.standard_normal(D) * 0.05).astype(np.float32),
    }
    got = kernel(**inputs)

    hs64 = inputs["hidden_states"].astype(np.float64)
    q = hs64 @ inputs["Wq"].astype(np.float64) + inputs["bq"].astype(np.float64)
    k = hs64 @ inputs["Wk"].astype(np.float64) + inputs["bk"].astype(np.float64)
    v = hs64 @ inputs["Wv"].astype(np.float64) + inputs["bv"].astype(np.float64)
    sc = np.einsum("bsd,btd->bst", q, k) / 8.0
    sc += inputs["attention_mask"].astype(np.float64)
    sc -= sc.max(axis=-1, keepdims=True)
    p = np.exp(sc)
    p /= p.sum(axis=-1, keepdims=True)
    ref = np.einsum("bst,btd->bsd", p, v)
    err = np.abs(got.astype(np.float64) - ref)
    print(
        f"absmax={err.max():.3e} rel_vs_scale={err.max() / np.abs(ref).max():.3e} "
        f"rms_rel={np.sqrt((err**2).mean()) / np.sqrt((ref**2).mean()):.3e}"
    )
